# revision 1
# baseline (speedup 1.0000x reference)
"""GATv2 molecular-graph kernel for 8 TRN2 NeuronCores (SPMD, data-parallel).

Host side: edges sorted by destination node, nodes partitioned into 8
contiguous ranges with ~equal edge counts.  Each core processes its node
range: edges are packed into chunks of <=256 edge slots covering <=127
consecutive nodes (node column 127 of each chunk is a trash column for
padding edges).  Per-core node tables are compacted to the union of
endpoint nodes so gather indices stay small.

Algebraic folds done on host:
  - BatchNorm (eval) folded into the encoder matmul:  h = relu(x @ We' + be')
    with the bias folded in via an appended all-ones feature column.
  - logits = sum_c att_c * lrelu(xe_c) = 0.6*(xe @ att) + 0.4*sum_c s_c*|xe_c * |att_c||
    so |att| is folded into Wl/Wr/We (columns sign-permuted so each head's
    block is [positive-att | negative-att]), and the linear term uses
    wl_att = 0.6 * Wl @ att per head.
  - Softmax denominators are applied per *node* after aggregation
    (alpha never materializes): agg_h = sum_e ex_e * h_src_e, then
    out24 = sum_h (agg_h @ (Wl_h @ Wp_h)) * recip(seg_sum_h)  (+ bias terms).
  - Edge biases (bl+br) ride an appended all-ones column of edge_attr.
  - Remaining constant terms (bl@Wp etc.) are added on host.
"""

import numpy as np

import concourse.bacc as bacc
import concourse.tile as tile
from concourse import mybir
import concourse.bass as bass
from concourse.bass_utils import run_bass_kernel_spmd
from concourse.masks import make_identity

P = 128
N_CORES = 8
CHUNK_E = 256          # edge slots per chunk (2 subtiles of 128)
CHUNK_N = 127          # max real nodes per chunk; col 127 = trash
NEG_SLOPE = 0.2
BN_EPS = 1e-5

FP = mybir.dt.float32
BF = mybir.dt.bfloat16
I32 = mybir.dt.int32


# ----------------------------------------------------------------------------
# host-side preparation
# ----------------------------------------------------------------------------

def _fold_weights(W_enc, b_enc, bn_gamma, bn_beta, bn_mean, bn_var,
                  Wl, bl, Wr, br, We, att, bias_conv, Wp, bp):
    D = W_enc.shape[1]
    H, C = att.shape
    HC = H * C
    s = bn_gamma / np.sqrt(bn_var + BN_EPS)
    W_enc_f = W_enc * s[None, :]
    b_enc_f = (b_enc - bn_mean) * s + bn_beta
    W_enc_aug = np.concatenate([W_enc_f, b_enc_f[None, :]], 0)  # [33, D]

    att_flat = att.reshape(HC)
    # sign permutation within each head block: positives first
    perm = np.zeros(HC, dtype=np.int64)
    pos_w = np.zeros(H, dtype=np.int64)
    for h in range(H):
        a = att[h]
        order = np.argsort((a <= 0).astype(np.int64), kind="stable")
        perm[h * C:(h + 1) * C] = h * C + order
        pos_w[h] = int((a > 0).sum())

    absatt = np.abs(att_flat[perm])
    Wl2 = (Wl[:, perm] * absatt[None, :])
    Wr2 = (Wr[:, perm] * absatt[None, :])
    We2 = (We[:, perm] * absatt[None, :])
    # edge-attr augmented with ones column carrying (bl + br)
    bsum = (bl + br)
    We2_aug = np.concatenate([We2, (bsum[perm] * absatt)[None, :]], 0)  # [17, HC]

    # linear logit term: 0.6 * (x_edge @ att) per head
    wla = 0.6 * np.stack([Wl[:, h * C:(h + 1) * C] @ att[h] for h in range(H)], 1)
    wra = 0.6 * np.stack([Wr[:, h * C:(h + 1) * C] @ att[h] for h in range(H)], 1)
    wea_ = 0.6 * np.stack([We[:, h * C:(h + 1) * C] @ att[h] for h in range(H)], 1)
    bea = 0.6 * np.stack([bsum[h * C:(h + 1) * C] @ att[h] for h in range(H)], 0)
    wea_aug = np.concatenate([wea_, bea[None, :]], 0)  # [17, H]

    # folded node transform: out24_h = agg_h @ (Wl_h @ Wp_h)
    OUT = Wp.shape[1]
    Wfold = np.concatenate(
        [Wl[:, h * C:(h + 1) * C] @ Wp[h * C:(h + 1) * C] for h in range(H)], 1
    )  # [D, H*OUT]

    # constants: out = ... + sgn*(bl@Wp) + (bias_conv@Wp + bp)
    cbl = bl @ Wp                      # [OUT] multiplied by 1{deg>0}
    cc = bias_conv @ Wp + bp           # [OUT] always
    wfa = np.concatenate([Wfold, wla], 1)  # [D, H*OUT + H]
    return dict(W_enc_aug=W_enc_aug, Wl2=Wl2, Wr2=Wr2, We2_aug=We2_aug,
                wla=wla, wra=wra, wea_aug=wea_aug, Wfold=Wfold, wfa=wfa,
                cbl=cbl, cc=cc, pos_w=pos_w, H=H, C=C, OUT=OUT, D=D)


def _prepare(x, edge_attr, edge_index, fw):
    """Shard + pack everything. Returns (in_maps, meta)."""
    N = x.shape[0]
    E = edge_index.shape[1]
    H, OUT = fw["H"], fw["OUT"]
    src = np.asarray(edge_index[0], dtype=np.int64)
    dst = np.asarray(edge_index[1], dtype=np.int64)

    order = np.argsort(dst, kind="stable")
    src_s = src[order]
    dst_s = dst[order]
    ea_s = np.asarray(edge_attr, dtype=np.float32)[order]

    deg = np.bincount(dst, minlength=N)
    cum = np.concatenate([[0], np.cumsum(deg)])  # edges before node n

    # node range boundaries: ~equal edges
    bounds = [0]
    for c in range(1, N_CORES):
        bounds.append(int(np.searchsorted(cum, E * c // N_CORES)))
    bounds.append(N)

    cores = []
    for c in range(N_CORES):
        n0, n1 = bounds[c], bounds[c + 1]
        e0, e1 = int(cum[n0]), int(cum[n1])
        # --- chunking: consecutive nodes, <=CHUNK_N real nodes, <=CHUNK_E edges
        chunks = []  # list of (na, nb) node ranges
        na = n0
        while na < n1:
            nb = na
            ecnt = 0
            while nb < n1 and (nb - na) < CHUNK_N and ecnt + deg[nb] <= CHUNK_E:
                ecnt += deg[nb]
                nb += 1
            if nb == na:          # single node with deg > CHUNK_E: cannot happen here
                raise RuntimeError("node degree exceeds chunk capacity")
            chunks.append((na, nb))
            na = nb
        nch = len(chunks)

        # --- per-chunk edge slots
        src_g = np.zeros((nch, CHUNK_E), dtype=np.int64)
        dst_g = np.zeros((nch, CHUNK_E), dtype=np.int64)
        dloc = np.full((nch, CHUNK_E), 127, dtype=np.float32)
        ea_p = np.zeros((nch, CHUNK_E, ea_s.shape[1] + 1), dtype=np.float32)
        real_nodes = np.zeros(nch, dtype=np.int64)
        for k, (a, b) in enumerate(chunks):
            ee0, ee1 = int(cum[a]), int(cum[b])
            m = ee1 - ee0
            src_g[k, :m] = src_s[ee0:ee1]
            dst_g[k, :m] = dst_s[ee0:ee1]
            dloc[k, :m] = (dst_s[ee0:ee1] - a).astype(np.float32)
            ea_p[k, :m, :-1] = ea_s[ee0:ee1]
            ea_p[k, :m, -1] = 1.0
            real_nodes[k] = b - a

        # --- compact node table
        used = np.zeros(N, dtype=bool)
        used[src_g.reshape(-1)] = True
        used[dst_g.reshape(-1)] = True
        needed = np.nonzero(used)[0]
        remap = np.zeros(N, dtype=np.int64)
        remap[needed] = np.arange(len(needed))
        src_t = remap[src_g]
        dst_t = remap[dst_g]
        x_need = np.asarray(x, dtype=np.float32)[needed]

        cores.append(dict(chunks=chunks, nch=nch, nu=len(needed),
                          src_t=src_t, dst_t=dst_t, dloc=dloc, ea_p=ea_p,
                          x_need=x_need, real_nodes=real_nodes))

    NCH = max(cd["nch"] for cd in cores)
    NU = max(cd["nu"] for cd in cores)
    NUpad = ((NU + P - 1) // P) * P

    in_maps = []
    for cd in cores:
        nch, nu = cd["nch"], cd["nu"]
        NS = NCH * 2
        # xT_aug [33, NUpad] bf16
        xt = np.zeros((33, NUpad), dtype=np.float32)
        xt[:32, :nu] = cd["x_need"].T
        xt[32, :nu] = 1.0
        # idx tiles [128, NS]
        def pack_idx(a):  # [nch, CHUNK_E] -> [128, NS]
            out = np.zeros((P, NS), dtype=np.int32)
            v = a.reshape(nch, 2, P).transpose(2, 0, 1).reshape(P, nch * 2)
            out[:, :nch * 2] = v
            return out
        src_i = pack_idx(cd["src_t"])
        dst_i = pack_idx(cd["dst_t"])
        dl = np.full((P, NS), 127.0, dtype=np.float32)
        dl[:, :nch * 2] = cd["dloc"].reshape(nch, 2, P).transpose(2, 0, 1).reshape(P, nch * 2)
        # eaT [17, NCH*CHUNK_E] bf16
        eat = np.zeros((17, NCH * CHUNK_E), dtype=np.float32)
        eat[:, :nch * CHUNK_E] = cd["ea_p"].reshape(nch * CHUNK_E, 17).T

        fw16 = lambda a: a.astype(np.float32)  # dram params stay f32-typed? -> cast below
        in_maps.append({
            "xt": xt.astype(np.float32),
            "src_idx": src_i,
            "dst_idx": dst_i,
            "dstloc": dl,
            "eat": eat.astype(np.float32),
        })

    meta = dict(NCH=NCH, NUpad=NUpad, cores=cores, bounds=bounds,
                H=H, OUT=OUT)
    return in_maps, meta


# ----------------------------------------------------------------------------
# device kernel builder
# ----------------------------------------------------------------------------

def _build(NCH, NUpad, fw):
    H, C, OUT, D = fw["H"], fw["C"], fw["OUT"], fw["D"]
    HC = H * C
    NS = NCH * 2
    NG = NUpad // P
    pos_w = fw["pos_w"]

    nc = bacc.Bacc("TRN2", target_bir_lowering=False, debug=False,
                   num_devices=N_CORES)

    # ---- dram I/O
    xt_d = nc.declare_dram_parameter("xt", [33, NUpad], FP, isOutput=False)
    src_d = nc.declare_dram_parameter("src_idx", [P, NS], I32, isOutput=False)
    dst_d = nc.declare_dram_parameter("dst_idx", [P, NS], I32, isOutput=False)
    dloc_d = nc.declare_dram_parameter("dstloc", [P, NS], FP, isOutput=False)
    eat_d = nc.declare_dram_parameter("eat", [17, NCH * CHUNK_E], FP, isOutput=False)
    out_d = nc.declare_dram_parameter("out", [NCH * P, OUT], FP, isOutput=True)

    htab = nc.dram_tensor("h_table", [NUpad, D], BF)

    # ---- constant weights baked into the NEFF as dram inputs
    wenc_d = nc.declare_dram_parameter("wenc", [33, D], FP, isOutput=False)
    wl2_d = nc.declare_dram_parameter("wl2", [D, HC], FP, isOutput=False)
    wr2_d = nc.declare_dram_parameter("wr2", [D, HC], FP, isOutput=False)
    we2_d = nc.declare_dram_parameter("we2", [17, HC], FP, isOutput=False)
    wra_d = nc.declare_dram_parameter("wra", [D, H], FP, isOutput=False)
    wea_d = nc.declare_dram_parameter("wea", [17, H], FP, isOutput=False)
    wfa_d = nc.declare_dram_parameter("wfa", [D, H * OUT + H], FP, isOutput=False)

    with tile.TileContext(nc) as tc:
        with (
            tc.tile_pool(name="const", bufs=1) as constp,
            tc.tile_pool(name="gath", bufs=3) as gathp,
            tc.tile_pool(name="gt", bufs=3) as gtp,
            tc.tile_pool(name="wide", bufs=3) as widep,
            tc.tile_pool(name="small", bufs=4) as smallp,
            tc.tile_pool(name="scr", bufs=4) as scrp,
            tc.tile_pool(name="outp", bufs=3) as outp,
            tc.tile_pool(name="ptr", bufs=2, space="PSUM") as ptr,      # transposes
            tc.tile_pool(name="px", bufs=2, space="PSUM") as px,        # x_edge
            tc.tile_pool(name="ptg", bufs=2, space="PSUM") as ptg,      # gfold+t1
            tc.tile_pool(name="pacc", bufs=2, space="PSUM") as pacc,    # seg+agg
        ):
            # ---- resident constants
            def load_const(dram, shape, dtype, nm):
                t = constp.tile(shape, dtype, tag=nm, name=nm)
                nc.sync.dma_start(out=t[:], in_=dram[:])
                return t
            # weight tiles (bf16 via dma cast on gpsimd)
            def load_const_bf(dram, shape, nm):
                t = constp.tile(shape, BF, tag=nm, name=nm)
                nc.gpsimd.dma_start(out=t[:], in_=dram[:])
                return t

            wenc = load_const_bf(wenc_d, [33, D], "wenc")
            wl2 = load_const_bf(wl2_d, [D, HC], "wl2")
            wr2 = load_const_bf(wr2_d, [D, HC], "wr2")
            we2 = load_const_bf(we2_d, [17, HC], "we2")
            wra = load_const_bf(wra_d, [D, H], "wra")
            wea = load_const_bf(wea_d, [17, H], "wea")
            wfa = load_const_bf(wfa_d, [D, H * OUT + H], "wfa")
            srci = load_const(src_d, [P, NS], I32, "srci")
            dsti = load_const(dst_d, [P, NS], I32, "dsti")
            dlocf = load_const(dloc_d, [P, NS], FP, "dlocf")

            ident = constp.tile([P, P], BF)
            make_identity(nc, ident[:])
            iota_i = constp.tile([P, P], I32)
            nc.gpsimd.iota(iota_i[:], pattern=[[1, P]], base=0,
                           channel_multiplier=0)
            iota_f = constp.tile([P, P], FP)
            nc.vector.tensor_copy(iota_f[:], iota_i[:])

            # ---- phase A: h table
            for g in range(NG):
                xtile = gathp.tile([33, P], BF, tag="xt")
                nc.gpsimd.dma_start(out=xtile[:], in_=xt_d[:, g * P:(g + 1) * P])
                hps = ptr.tile([P, D], FP, tag="tp", name="hps")
                nc.tensor.matmul(hps[:], lhsT=xtile[:], rhs=wenc[:],
                                 start=True, stop=True)
                hsb = gtp.tile([P, D], BF, tag="hsb")
                nc.scalar.activation(hsb[:], hps[:],
                                     mybir.ActivationFunctionType.Relu)
                nc.sync.dma_start(out=htab[g * P:(g + 1) * P, :], in_=hsb[:])

            # ---- phase B: edges
            W96 = 4 * OUT              # gfold width (H*OUT)
            W100 = W96 + H             # + t1 columns
            for k in range(NCH):
                ea_sb = gathp.tile([17, CHUNK_E], BF, tag="ea", name=f"ea_{k}")
                nc.gpsimd.dma_start(
                    out=ea_sb[:], in_=eat_d[:, k * CHUNK_E:(k + 1) * CHUNK_E])

                acc_ps = pacc.tile([P, W100], FP, tag="acc", name=f"accps_{k}")

                for s in range(2):
                    col = k * 2 + s
                    gs = gathp.tile([P, D], BF, tag="gs", name=f"gs_{col}")
                    nc.gpsimd.indirect_dma_start(
                        out=gs[:], out_offset=None, in_=htab[:],
                        in_offset=bass.IndirectOffsetOnAxis(
                            ap=srci[:, col:col + 1], axis=0))
                    gd = gathp.tile([P, D], BF, tag="gd", name=f"gd_{col}")
                    nc.gpsimd.indirect_dma_start(
                        out=gd[:], out_offset=None, in_=htab[:],
                        in_offset=bass.IndirectOffsetOnAxis(
                            ap=dsti[:, col:col + 1], axis=0))

                    # transposes
                    tps = ptr.tile([P, P], BF, tag="tp", name=f"tps_{col}")
                    nc.tensor.transpose(tps[:], gs[:], ident[:])
                    gst = gtp.tile([P, P], BF, tag="gst", name=f"gst_{col}")
                    nc.vector.tensor_copy(gst[:], tps[:])
                    tpd = ptr.tile([P, P], BF, tag="tp", name=f"tpd_{col}")
                    nc.tensor.transpose(tpd[:], gd[:], ident[:])
                    gdt = gtp.tile([P, P], BF, tag="gdt", name=f"gdt_{col}")
                    nc.scalar.activation(gdt[:], tpd[:],
                                         mybir.ActivationFunctionType.Copy)

                    # x_edge [e, HC], gfold+t1 [e, 100]
                    X = px.tile([P, HC], FP, tag="X", name=f"X_{col}")
                    tg = ptg.tile([P, W100], FP, tag="tg", name=f"tg_{col}")
                    ea_sl = ea_sb[:, s * P:(s + 1) * P]
                    nc.tensor.matmul(X[:], lhsT=gst[:], rhs=wl2[:],
                                     start=True, stop=False)
                    nc.tensor.matmul(tg[:], lhsT=gst[:], rhs=wfa[:],
                                     start=True, stop=False)
                    nc.tensor.matmul(X[:], lhsT=gdt[:], rhs=wr2[:],
                                     start=False, stop=False)
                    nc.tensor.matmul(tg[:, W96:W100], lhsT=gdt[:], rhs=wra[:],
                                     start=False, stop=False, skip_group_check=True)
                    nc.tensor.matmul(X[:], lhsT=ea_sl, rhs=we2[:],
                                     start=False, stop=True)
                    nc.tensor.matmul(tg[:, W96:W100], lhsT=ea_sl, rhs=wea[:],
                                     start=False, stop=True, skip_group_check=True)

                    # signed abs reduction: pos block -> DVE, neg block -> ACT
                    acc8 = smallp.tile([P, 2 * H], FP, tag="acc8", name=f"acc8_{col}")
                    scratch = scrp.tile([P, P], FP, tag="scr", name=f"scr_{col}")
                    for h in range(H):
                        pw = int(pos_w[h])
                        if pw > 0:
                            nc.vector.tensor_reduce(
                                acc8[:, 2 * h:2 * h + 1],
                                X[:, h * C:h * C + pw],
                                axis=mybir.AxisListType.X,
                                op=mybir.AluOpType.add,
                                apply_absolute_value=True)
                        else:
                            nc.vector.memset(acc8[:, 2 * h:2 * h + 1], 0.0)
                        if pw < C:
                            nc.scalar.activation(
                                scratch[:, :C - pw],
                                X[:, h * C + pw:(h + 1) * C],
                                mybir.ActivationFunctionType.Abs,
                                accum_out=acc8[:, 2 * h + 1:2 * h + 2])
                        else:
                            nc.vector.memset(acc8[:, 2 * h + 1:2 * h + 2], 0.0)

                    # logits -> ex
                    t1s = smallp.tile([P, H], FP, tag="t1s", name=f"t1s_{col}")
                    nc.scalar.activation(t1s[:], tg[:, W96:W100],
                                         mybir.ActivationFunctionType.Copy)
                    df = smallp.tile([P, H], FP, tag="df", name=f"df_{col}")
                    nc.vector.tensor_tensor(
                        out=df[:], in0=acc8[:, 0:2 * H:2], in1=acc8[:, 1:2 * H:2],
                        op=mybir.AluOpType.subtract)
                    exf = smallp.tile([P, H], FP, tag="exf", name=f"exf_{col}")
                    for h in range(H):
                        nc.scalar.activation(
                            exf[:, h:h + 1], df[:, h:h + 1],
                            mybir.ActivationFunctionType.Exp,
                            scale=0.4, bias=t1s[:, h:h + 1])

                    # S [e, n] indicator
                    S = gtp.tile([P, P], BF, tag="S", name=f"S_{col}")
                    nc.vector.tensor_tensor(
                        out=S[:], in0=dlocf[:, col:col + 1].to_broadcast([P, P]),
                        in1=iota_f[:], op=mybir.AluOpType.is_equal)

                    # gf = [gfold*ex | ex]  [e, 100] bf16
                    gf = widep.tile([P, W100], BF, tag="gf", name=f"gf_{col}")
                    for h in range(H):
                        nc.vector.tensor_scalar(
                            out=gf[:, h * OUT:(h + 1) * OUT],
                            in0=tg[:, h * OUT:(h + 1) * OUT],
                            scalar1=exf[:, h:h + 1], scalar2=None,
                            op0=mybir.AluOpType.mult)
                    nc.vector.tensor_copy(gf[:, W96:W100], exf[:])

                    # aggregate: acc [n, 100] += S.T @ gf
                    nc.tensor.matmul(acc_ps[:], lhsT=S[:], rhs=gf[:],
                                     start=(s == 0), stop=(s == 1))

                # ---- chunk finalize
                srec = smallp.tile([P, H], FP, tag="srec", name=f"srec_{k}")
                nc.vector.tensor_scalar_add(srec[:], acc_ps[:, W96:W100], 1e-6)
                rec = smallp.tile([P, H], FP, tag="rec", name=f"rec_{k}")
                nc.vector.reciprocal(rec[:], srec[:])

                m = []
                for h in range(H):
                    mh = outp.tile([P, OUT], FP, tag=f"m{h}", name=f"m{h}_{k}")
                    nc.vector.tensor_scalar(
                        out=mh[:], in0=acc_ps[:, h * OUT:(h + 1) * OUT],
                        scalar1=rec[:, h:h + 1], scalar2=None,
                        op0=mybir.AluOpType.mult)
                    m.append(mh)
                o01 = outp.tile([P, OUT], FP, tag="o01", name=f"o01_{k}")
                o23 = outp.tile([P, OUT], FP, tag="o23", name=f"o23_{k}")
                nc.vector.tensor_tensor(out=o01[:], in0=m[0][:], in1=m[1][:],
                                        op=mybir.AluOpType.add)
                nc.vector.tensor_tensor(out=o23[:], in0=m[2][:], in1=m[3][:],
                                        op=mybir.AluOpType.add)
                o = outp.tile([P, OUT], FP, tag="o", name=f"o_{k}")
                nc.vector.tensor_tensor(out=o[:], in0=o01[:], in1=o23[:],
                                        op=mybir.AluOpType.add)
                nc.sync.dma_start(out=out_d[k * P:(k + 1) * P, :], in_=o[:])

    nc.compile()
    return nc


# ----------------------------------------------------------------------------
# public entry
# ----------------------------------------------------------------------------

_CACHE = {}
LAST_RUN = {}


def _run(x, edge_attr, edge_index, W_enc, b_enc, bn_gamma, bn_beta, bn_mean,
         bn_var, Wl, bl, Wr, br, We, att, bias_conv, Wp, bp):
    x = np.asarray(x)
    fw = _fold_weights(np.asarray(W_enc, np.float32), np.asarray(b_enc, np.float32),
                       np.asarray(bn_gamma, np.float32), np.asarray(bn_beta, np.float32),
                       np.asarray(bn_mean, np.float32), np.asarray(bn_var, np.float32),
                       np.asarray(Wl, np.float32), np.asarray(bl, np.float32),
                       np.asarray(Wr, np.float32), np.asarray(br, np.float32),
                       np.asarray(We, np.float32), np.asarray(att, np.float32),
                       np.asarray(bias_conv, np.float32), np.asarray(Wp, np.float32),
                       np.asarray(bp, np.float32))
    in_maps, meta = _prepare(x, edge_attr, edge_index, fw)
    NCH, NUpad = meta["NCH"], meta["NUpad"]

    key = (NCH, NUpad, tuple(fw["pos_w"].tolist()))
    if key not in _CACHE:
        _CACHE[key] = _build(NCH, NUpad, fw)
    nc = _CACHE[key]

    wmap = {
        "wenc": fw["W_enc_aug"].astype(np.float32),
        "wl2": fw["Wl2"].astype(np.float32),
        "wr2": fw["Wr2"].astype(np.float32),
        "we2": fw["We2_aug"].astype(np.float32),
        "wra": fw["wra"].astype(np.float32),
        "wea": fw["wea_aug"].astype(np.float32),
        "wfa": fw["wfa"].astype(np.float32),
    }
    for im in in_maps:
        im.update(wmap)

    LAST_RUN["in_maps"] = in_maps
    LAST_RUN["nc"] = nc
    res = run_bass_kernel_spmd(nc, in_maps, core_ids=list(range(N_CORES)))

    # ---- unshard
    N = x.shape[0]
    OUT = fw["OUT"]
    out = np.zeros((N, OUT), dtype=np.float32)
    H = fw["H"]
    for c, cd in enumerate(meta["cores"]):
        dev = res.results[c]["out"]          # [NCH*128, OUT]
        for k, (a, b) in enumerate(cd["chunks"]):
            out[a:b] = dev[k * P:k * P + (b - a)]
    # host-side constant terms
    cc = fw["cc"]
    cbl = fw["cbl"]
    if np.any(cc != 0) or np.any(cbl != 0):
        deg = np.bincount(np.asarray(edge_index[1], np.int64), minlength=N)
        sgn = (deg > 0).astype(np.float32)[:, None]
        out = out + sgn * cbl[None, :] + cc[None, :]
    return out


def kernel(**inputs):
    out = _run(
        inputs["x"], inputs["edge_attr"], inputs["edge_index"],
        inputs["W_enc"], inputs["b_enc"], inputs["bn_gamma"], inputs["bn_beta"],
        inputs["bn_mean"], inputs["bn_var"], inputs["Wl"], inputs["bl"],
        inputs["Wr"], inputs["br"], inputs["We"], inputs["att"],
        inputs["bias_conv"], inputs["Wp"], inputs["bp"])
    return out.astype(np.float32)



# revision 10
# speedup vs baseline: 2.8691x; 2.8691x over previous
"""GATv2 molecular-graph kernel for 8 TRN2 NeuronCores (SPMD, data-parallel).

Host side (layout only — all reference FLOPs run on device): edges are
sorted by destination node and partitioned into 8 contiguous ranges with
~equal edge counts.  Per core, edges are packed into chunks of <=256 edge
slots covering <=128 distinct destination nodes; two chunks form a "pair"
(512 edge slots) which is the device work unit.  For each edge slot the
host gathers the RAW inputs (x[src], x[dst], edge_attr) into dense bf16
tensors, so the device needs no indirect DMA at all.

Device pipeline per pair (transposed feature-major layout):
  hsT/hdT = relu(wencA^T @ xsdT)            encoder (bn folded, bias via
                                            appended all-ones feature row)
  X^T_h   = wl2_h^T hs + wr2_h^T hd + we2_h^T ea    [C=128, 512 edges]
            (weights pre-scaled by 0.4*|att|, col-signs NOT permuted)
  A_h     = |X^T_h|                          (scalar/vector engines)
  dfq     = t1 (3 small matmuls) + sum_c sign(att_c)*A_h  (4 sigma-matmuls
            with single-column lhsT landing on PSUM partitions 0/32/64/96)
            = full GATv2 logits: 0.6*(z@att) + 0.4*sum|z_c att_c|
  ex      = exp(dfq)  (no segment-max shift; logits are O(0.1))
  gf      = [gfold*ex | ex]  where gfold = hs @ (Wl_h @ Wp_h)  [e,96]
  acc     = S^T @ gf  per subtile (S = slot-indicator built on gpsimd)
  out     = sum_h acc_h * 1/(denom_h)       (vector engine)
Softmax denominators ride the last 4 columns of gf.  Host adds the
constant terms (bias_conv@Wp + bp, and bl@Wp gated by deg>0).
"""

import numpy as np
import ml_dtypes

import concourse.bacc as bacc
import concourse.tile as tile
from concourse import mybir
import concourse.bass as bass
from concourse.bass_utils import run_bass_kernel_spmd
from concourse.masks import make_identity

P = 128
N_CORES = 8
EPC = 256            # edge slots per chunk
EPP = 512            # edge slots per pair (2 chunks)
SLOTS = 128          # max distinct dst nodes per chunk
NEG_SLOPE = 0.2
BN_EPS = 1e-5

FP = mybir.dt.float32
BF = mybir.dt.bfloat16
I32 = mybir.dt.int32
BF_NP = ml_dtypes.bfloat16


# ----------------------------------------------------------------------------
# host-side weight folding
# ----------------------------------------------------------------------------

def _fold_weights(W_enc, b_enc, bn_gamma, bn_beta, bn_mean, bn_var,
                  Wl, bl, Wr, br, We, att, bias_conv, Wp, bp):
    D = W_enc.shape[1]
    H, C = att.shape
    HC = H * C
    OUT = Wp.shape[1]
    s = bn_gamma / np.sqrt(bn_var + BN_EPS)
    W_enc_f = W_enc * s[None, :]
    b_enc_f = (b_enc - bn_mean) * s + bn_beta
    wencA = np.concatenate([W_enc_f, b_enc_f[None, :]], 0)      # [33, D]

    att_abs = np.abs(att)                                       # [H, C]
    bsum = bl + br                                              # [HC]

    # abs-path weights: X = 0.4 * |att| * z   (z = hs@Wl + hd@Wr + ea@We + b)
    wl2 = np.zeros((D, HC), np.float32)
    wr2 = np.zeros((D, HC), np.float32)
    we2 = np.zeros((17, HC), np.float32)
    for h in range(H):
        blk = slice(h * C, (h + 1) * C)
        wl2[:, blk] = 0.4 * Wl[:, blk] * att_abs[h][None, :]
        wr2[:, blk] = 0.4 * Wr[:, blk] * att_abs[h][None, :]
        we2[:16, blk] = 0.4 * We[:, blk] * att_abs[h][None, :]
        we2[16, blk] = 0.4 * bsum[blk] * att_abs[h]

    sgm = np.where(att > 0, 1.0, -1.0).astype(np.float32).T     # [C, H]

    # linear logit path: t1 = 0.6 * (z @ att_h), landing on partition 32h
    wla = np.zeros((D, 97), np.float32)
    wra = np.zeros((D, 97), np.float32)
    wea = np.zeros((17, 97), np.float32)
    for h in range(H):
        blk = slice(h * C, (h + 1) * C)
        wla[:, 32 * h] = 0.6 * (Wl[:, blk] @ att[h])
        wra[:, 32 * h] = 0.6 * (Wr[:, blk] @ att[h])
        wea[:16, 32 * h] = 0.6 * (We[:, blk] @ att[h])
        wea[16, 32 * h] = 0.6 * (bsum[blk] @ att[h])

    # folded node->out transform per head
    wfold = np.concatenate(
        [Wl[:, h * C:(h + 1) * C] @ Wp[h * C:(h + 1) * C] for h in range(H)], 1
    )                                                           # [D, 96]

    cbl = bl @ Wp                       # [OUT]: * 1{deg>0}
    cc = bias_conv @ Wp + bp            # [OUT]: always
    return dict(wencA=wencA, wl2=wl2, wr2=wr2, we2=we2, sgm=sgm,
                wla=wla, wra=wra, wea=wea, wfold=wfold,
                cbl=cbl, cc=cc, H=H, C=C, OUT=OUT, D=D)


# ----------------------------------------------------------------------------
# host-side edge packing
# ----------------------------------------------------------------------------

def _prepare(x, edge_attr, edge_index):
    N = x.shape[0]
    E = edge_index.shape[1]
    src = np.asarray(edge_index[0], dtype=np.int64)
    dst = np.asarray(edge_index[1], dtype=np.int64)

    order = np.argsort(dst, kind="stable")
    src_s = src[order]
    dst_s = dst[order]
    ea_s = np.asarray(edge_attr, dtype=np.float32)[order]

    deg = np.bincount(dst, minlength=N)
    cum = np.concatenate([[0], np.cumsum(deg)])

    bounds = [0]
    for c in range(1, N_CORES):
        bounds.append(int(np.searchsorted(cum, E * c // N_CORES)))
    bounds.append(N)

    xf = np.asarray(x, dtype=np.float32)

    cores = []
    for c in range(N_CORES):
        n0, n1 = bounds[c], bounds[c + 1]
        # chunking: walk deg>0 nodes; <=SLOTS nodes and <=EPC edges per chunk
        chunks = []          # list of (list-of-node-ids, e_start, e_end)
        cur_nodes = []
        ce0 = int(cum[n0])
        ecnt = 0
        for n in range(n0, n1):
            d = int(deg[n])
            if d == 0:
                continue
            if d > EPC:
                raise RuntimeError("node degree exceeds chunk capacity")
            if len(cur_nodes) >= SLOTS or ecnt + d > EPC:
                chunks.append((cur_nodes, ce0, ce0 + ecnt))
                ce0 += ecnt
                cur_nodes = []
                ecnt = 0
            cur_nodes.append(n)
            ecnt += d
        if cur_nodes:
            chunks.append((cur_nodes, ce0, ce0 + ecnt))
        cores.append(dict(chunks=chunks, n0=n0, n1=n1))

    NCH = max(len(cd["chunks"]) for cd in cores)
    NP = (NCH + 1) // 2

    in_maps = []
    for cd in cores:
        chunks = cd["chunks"]
        # layout per pair p: [src slots (512) | dst slots (512)]
        xsd = np.zeros((33, NP * 1024), np.float32)
        ea17 = np.zeros((17, NP * EPP), np.float32)
        dloc = np.full((P, NP * 4), 999.0, np.float32)
        for k, (nodes, e0, e1) in enumerate(chunks):
            p, ci = divmod(k, 2)
            m = e1 - e0
            base = p * 1024 + ci * EPC
            sl = slice(base, base + m)
            xsd[:32, sl] = xf[src_s[e0:e1]].T
            xsd[32, sl] = 1.0
            sl2 = slice(base + 512, base + 512 + m)
            xsd[:32, sl2] = xf[dst_s[e0:e1]].T
            xsd[32, sl2] = 1.0
            eb = p * EPP + ci * EPC
            ea17[:16, eb:eb + m] = ea_s[e0:e1].T
            ea17[16, eb:eb + m] = 1.0
            # slot index of each edge's dst within the chunk node list
            node_arr = np.asarray(nodes)
            slot_of = {n: i for i, n in enumerate(nodes)}
            dl = np.array([slot_of[n] for n in dst_s[e0:e1]], np.float32)
            # dloc columns: pair p has 4 subtiles (ci*2 + sub)
            full = np.full(EPC, 999.0, np.float32)
            full[:m] = dl
            dloc[:, p * 4 + ci * 2] = full[:P]
            dloc[:, p * 4 + ci * 2 + 1] = full[P:]
        in_maps.append({
            "xsd": xsd.astype(BF_NP),
            "ea17": ea17.astype(BF_NP),
            "dloc": dloc,
        })

    meta = dict(NP=NP, cores=cores, bounds=bounds)
    return in_maps, meta


# ----------------------------------------------------------------------------
# device kernel builder
# ----------------------------------------------------------------------------

def _build(NP):
    nc = bacc.Bacc("TRN2", target_bir_lowering=False, debug=False,
                   num_devices=N_CORES)

    xsd_d = nc.declare_dram_parameter("xsd", [33, NP * 1024], BF, isOutput=False)
    ea_d = nc.declare_dram_parameter("ea17", [17, NP * EPP], BF, isOutput=False)
    dloc_d = nc.declare_dram_parameter("dloc", [P, NP * 4], FP, isOutput=False)
    out_d = nc.declare_dram_parameter("out", [NP * 2 * P, 24], FP, isOutput=True)

    wencA_d = nc.declare_dram_parameter("wencA", [33, P], FP, isOutput=False)
    wl2_d = nc.declare_dram_parameter("wl2", [P, 512], FP, isOutput=False)
    wr2_d = nc.declare_dram_parameter("wr2", [P, 512], FP, isOutput=False)
    we2_d = nc.declare_dram_parameter("we2", [17, 512], FP, isOutput=False)
    sgm_d = nc.declare_dram_parameter("sgm", [P, 4], FP, isOutput=False)
    wla_d = nc.declare_dram_parameter("wla", [P, 97], FP, isOutput=False)
    wra_d = nc.declare_dram_parameter("wra", [P, 97], FP, isOutput=False)
    wea_d = nc.declare_dram_parameter("wea", [17, 97], FP, isOutput=False)
    wfold_d = nc.declare_dram_parameter("wfold", [P, 96], FP, isOutput=False)

    with tile.TileContext(nc) as tc:
        with (
            tc.tile_pool(name="const", bufs=1) as constp,
            tc.tile_pool(name="gath", bufs=3) as gathp,
            tc.tile_pool(name="hsp", bufs=2) as hsp,
            tc.tile_pool(name="apool", bufs=2) as apool,
            tc.tile_pool(name="expool", bufs=2) as expool,
            tc.tile_pool(name="spool", bufs=2) as spool,
            tc.tile_pool(name="gfp", bufs=2) as gfp,
            tc.tile_pool(name="finp", bufs=2) as finp,
            tc.tile_pool(name="pph", bufs=2, space="PSUM") as pph,
            tc.tile_pool(name="pxp", bufs=2, space="PSUM") as pxp,
            tc.tile_pool(name="pdq", bufs=1, space="PSUM") as pdq,
            tc.tile_pool(name="ptg", bufs=1, space="PSUM") as ptg,
            tc.tile_pool(name="ptps", bufs=1, space="PSUM") as ptps,
            tc.tile_pool(name="pacc", bufs=1, space="PSUM") as pacc,
        ):
            def load_const_bf(dram, shape, nm):
                t = constp.tile(shape, BF, tag=nm, name=nm)
                nc.gpsimd.dma_start(out=t[:], in_=dram[:])
                return t

            wencA = load_const_bf(wencA_d, [33, P], "wencA")
            wl2 = load_const_bf(wl2_d, [P, 512], "wl2")
            wr2 = load_const_bf(wr2_d, [P, 512], "wr2")
            we2 = load_const_bf(we2_d, [17, 512], "we2")
            sgm = load_const_bf(sgm_d, [P, 4], "sgm")
            wla = load_const_bf(wla_d, [P, 97], "wla")
            wra = load_const_bf(wra_d, [P, 97], "wra")
            wea = load_const_bf(wea_d, [17, 97], "wea")
            wfold = load_const_bf(wfold_d, [P, 96], "wfold")

            dlocR = constp.tile([P, NP * 4], FP, name="dlocR")
            nc.sync.dma_start(out=dlocR[:], in_=dloc_d[:])

            ident = constp.tile([P, P], BF, name="ident")
            make_identity(nc, ident[:])
            iota_i = constp.tile([P, P], I32, name="iota_i")
            nc.gpsimd.iota(iota_i[:], pattern=[[1, P]], base=0,
                           channel_multiplier=0)
            iota_f = constp.tile([P, P], FP, name="iota_f")
            nc.vector.tensor_copy(iota_f[:], iota_i[:])

            state = {}

            def front(p):
                gx = gathp.tile([33, 1024], BF, tag="gx", name=f"gx_{p}")
                nc.gpsimd.dma_start(out=gx[:], in_=xsd_d[:, p * 1024:(p + 1) * 1024])
                ge = gathp.tile([17, EPP], BF, tag="ge", name=f"ge_{p}")
                nc.gpsimd.dma_start(out=ge[:], in_=ea_d[:, p * EPP:(p + 1) * EPP])

                # encoder
                ph_s = pph.tile([P, 512], FP, tag="ph", name=f"phs_{p}")
                nc.tensor.matmul(ph_s[:], lhsT=wencA[:], rhs=gx[:, 0:512],
                                 start=True, stop=True)
                hs = hsp.tile([P, 512], BF, tag="hs", name=f"hs_{p}")
                nc.scalar.activation(hs[:], ph_s[:],
                                     mybir.ActivationFunctionType.Relu)
                ph_d = pph.tile([P, 512], FP, tag="ph", name=f"phd_{p}")
                nc.tensor.matmul(ph_d[:], lhsT=wencA[:], rhs=gx[:, 512:1024],
                                 start=True, stop=True)
                hd = hsp.tile([P, 512], BF, tag="hd", name=f"hd_{p}")
                nc.vector.tensor_scalar(out=hd[:], in0=ph_d[:], scalar1=0.0,
                                        scalar2=None, op0=mybir.AluOpType.max)

                # X^T per head + abs
                A = apool.tile([P, 2048], BF, tag="A", name=f"A_{p}")
                for h in range(4):
                    px = pxp.tile([P, 512], FP, tag="px", name=f"px_{p}_{h}")
                    nc.tensor.matmul(px[:], lhsT=wl2[:, h * P:(h + 1) * P],
                                     rhs=hs[:], start=True, stop=False)
                    nc.tensor.matmul(px[:], lhsT=wr2[:, h * P:(h + 1) * P],
                                     rhs=hd[:], start=False, stop=False)
                    nc.tensor.matmul(px[:], lhsT=we2[:, h * P:(h + 1) * P],
                                     rhs=ge[:], start=False, stop=True)
                    asl = A[:, h * 512:(h + 1) * 512]
                    nc.scalar.activation(asl, px[:],
                                         mybir.ActivationFunctionType.Abs)

                # logits: t1 (partitions 32h) + signed abs sums
                dfq = pdq.tile([P, 512], FP, tag="dfq", name=f"dfq_{p}")
                nc.tensor.matmul(dfq[0:97, :], lhsT=wla[:], rhs=hs[:],
                                 start=True, stop=False)
                nc.tensor.matmul(dfq[0:97, :], lhsT=wra[:], rhs=hd[:],
                                 start=False, stop=False)
                nc.tensor.matmul(dfq[0:97, :], lhsT=wea[:], rhs=ge[:],
                                 start=False, stop=False)
                for h in range(4):
                    nc.tensor.matmul(dfq[32 * h:32 * h + 1, :],
                                     lhsT=sgm[:, h:h + 1],
                                     rhs=A[:, h * 512:(h + 1) * 512],
                                     start=False, stop=(h == 3),
                                     skip_group_check=True,
                                     tile_position=(0, 32 * h))

                ex = expool.tile([P, 512], BF, tag="ex", name=f"ex_{p}")
                nc.scalar.activation(ex[0:97, :], dfq[0:97, :],
                                     mybir.ActivationFunctionType.Exp)

                # S indicator (DVE; Pool lacks TensorTensor on TRN2)
                S = spool.tile([P, 512], BF, tag="S", name=f"S_{p}")
                nc.vector.tensor_tensor(
                    out=S[:].rearrange("p (s n) -> p s n", s=4),
                    in0=dlocR[:, 4 * p:4 * p + 4].to_broadcast([P, 4, P]),
                    in1=iota_f[:].unsqueeze(1).to_broadcast([P, 4, P]),
                    op=mybir.AluOpType.is_equal)

                state[p] = dict(hs=hs, ge=ge, ex=ex, S=S)

            def tail(p):
                st = state.pop(p)
                hs, ex, S = st["hs"], st["ex"], st["S"]

                # ex transposed back to edge-major: tps[:, s, 32h] = ex_h
                # (slots padded to 100 cols => 200B, 4-byte aligned)
                tps = ptps.tile([P, 4, 100], BF, tag="tps", name=f"tps_{p}")
                for s in range(4):
                    nc.tensor.transpose(tps[:, s, 0:97],
                                        ex[0:97, s * P:(s + 1) * P],
                                        ident[0:97, 0:97])

                # gfold
                tg = ptg.tile([P, 4, 96], FP, tag="tg", name=f"tg_{p}")
                for s in range(4):
                    nc.tensor.matmul(tg[:, s, :],
                                     lhsT=hs[:, s * P:(s + 1) * P],
                                     rhs=wfold[:], start=True, stop=True)

                # gf = [gfold * ex | ex]; ex lands in SBUF first so the
                # multiply reads only one PSUM operand (tg)
                gf = gfp.tile([P, 4, 100], BF, tag="gf", name=f"gf_{p}")
                nc.vector.tensor_copy(gf[:, :, 96:100], tps[:, :, 0:97:32])
                exb = gf[:, :, 96:100].unsqueeze(3).to_broadcast([P, 4, 4, 24])
                nc.vector.tensor_tensor(
                    out=gf[:, :, 0:96].rearrange("p s (h j) -> p s h j", h=4),
                    in0=tg[:].rearrange("p s (h j) -> p s h j", h=4),
                    in1=exb, op=mybir.AluOpType.mult)

                # aggregate per subtile into per-chunk acc
                acc = pacc.tile([P, 2, 100], FP, tag="acc", name=f"acc_{p}")
                for s in range(4):
                    nc.tensor.matmul(acc[:, s // 2, :],
                                     lhsT=S[:, s * P:(s + 1) * P],
                                     rhs=gf[:, s, :],
                                     start=(s % 2 == 0), stop=(s % 2 == 1))

                # finalize both chunks
                srec = finp.tile([P, 2, 4], FP, tag="srec", name=f"srec_{p}")
                nc.vector.tensor_scalar_add(srec[:], acc[:, :, 96:100], 1e-6)
                rec = finp.tile([P, 2, 4], FP, tag="rec", name=f"rec_{p}")
                nc.vector.reciprocal(rec[:], srec[:])
                os = finp.tile([P, 2, 24, 4], FP, tag="os", name=f"os_{p}")
                nc.vector.tensor_tensor(
                    out=os[:].rearrange("p c j h -> p c h j"),
                    in0=acc[:, :, 0:96].rearrange("p c (h j) -> p c h j", h=4),
                    in1=rec[:].unsqueeze(3).to_broadcast([P, 2, 4, 24]),
                    op=mybir.AluOpType.mult)
                o2 = finp.tile([P, 48], FP, tag="o2", name=f"o2_{p}")
                nc.vector.tensor_reduce(
                    out=o2[:].rearrange("p (c j) -> p c j", c=2),
                    in_=os[:], axis=mybir.AxisListType.X,
                    op=mybir.AluOpType.add)
                for ci in range(2):
                    k = 2 * p + ci
                    nc.sync.dma_start(out=out_d[k * P:(k + 1) * P, :],
                                      in_=o2[:, ci * 24:(ci + 1) * 24])

            for p in range(NP):
                front(p)
                if p >= 1:
                    tail(p - 1)
            tail(NP - 1)

    nc.compile()
    return nc


# ----------------------------------------------------------------------------
# public entry
# ----------------------------------------------------------------------------

_CACHE = {}
LAST_RUN = {}


def kernel(**inputs):
    x = np.asarray(inputs["x"])
    edge_attr = np.asarray(inputs["edge_attr"])
    edge_index = np.asarray(inputs["edge_index"])
    fw = _fold_weights(
        *[np.asarray(inputs[k], np.float32) for k in
          ("W_enc", "b_enc", "bn_gamma", "bn_beta", "bn_mean", "bn_var",
           "Wl", "bl", "Wr", "br", "We", "att", "bias_conv", "Wp", "bp")])

    in_maps, meta = _prepare(x, edge_attr, edge_index)
    NP = meta["NP"]

    if NP not in _CACHE:
        _CACHE[NP] = _build(NP)
    nc = _CACHE[NP]

    wmap = {
        "wencA": fw["wencA"].astype(np.float32),
        "wl2": fw["wl2"].astype(np.float32),
        "wr2": fw["wr2"].astype(np.float32),
        "we2": fw["we2"].astype(np.float32),
        "sgm": fw["sgm"].astype(np.float32),
        "wla": fw["wla"].astype(np.float32),
        "wra": fw["wra"].astype(np.float32),
        "wea": fw["wea"].astype(np.float32),
        "wfold": fw["wfold"].astype(np.float32),
    }
    for im in in_maps:
        im.update(wmap)

    LAST_RUN["in_maps"] = in_maps
    LAST_RUN["nc"] = nc
    res = run_bass_kernel_spmd(nc, in_maps, core_ids=list(range(N_CORES)))

    # unshard
    N = x.shape[0]
    OUT = fw["OUT"]
    out = np.zeros((N, OUT), dtype=np.float32)
    for c, cd in enumerate(meta["cores"]):
        dev = np.asarray(res.results[c]["out"], np.float32)   # [NP*256, 24]
        for k, (nodes, e0, e1) in enumerate(cd["chunks"]):
            out[np.asarray(nodes)] = dev[k * P:k * P + len(nodes)]

    deg = np.bincount(np.asarray(edge_index[1], np.int64), minlength=N)
    sgn = (deg > 0).astype(np.float32)[:, None]
    out = out + sgn * fw["cbl"][None, :] + fw["cc"][None, :]
    return out.astype(np.float32)
